# revision 23
# baseline (speedup 1.0000x reference)
"""Trainium2 Bass kernel v4 for the dense transformer block (B=4,T=2048,C=1024,H=16,F=4096).

Sharding (head-parallel attention + Megatron MLP across core pairs):
- core 2s+p owns sequence s; parity p owns heads 8p..8p+8 and computes
  K/Q/V + scores/softmax/AV for those heads over ALL T tokens. Every core
  sees the same symmetric causal structure: chunk ci (512 q) needs exactly
  4(ci+1) k-tiles, the last 4 straddling the diagonal -> one shared
  [P,4,CH] byte mask, zero wasted score/exp work, no parity masks.
- proj is row-split by heads: each core computes a partial [C, 512] from its
  own y; partials are summed with a pairwise DRAM AllReduce. Both cores then
  run LN2 (duplicated, on (x + proj)/2 with eps/4) and a Megatron F-split
  fc1/fc2 (each core holds HALF of w_fc1/w_fc2 resident in SBUF - no weight
  streaming). fc2 partials + xo/2 go through a pairwise ReduceScatter(add)
  which lands each core's q-quarter of the final output, DMA'd DRAM->DRAM
  to the output tensor.
- Scores/exp masking via uint16 bitwise-AND on byte masks (DVE 2x mode).
- MLP(ci) is emitted as PE filler micro-blocks between attention score
  groups of ci+1; attention ci0 is interleaved into the LN1/QKV phase.
"""
import sys, types
import numpy as np
import ml_dtypes


def _install_hooks():
    try:
        import antenv
        if "antenv.axon_hooks" not in sys.modules:
            m = types.ModuleType("antenv.axon_hooks")
            m._hook = None
            m.set_axon_ntff_profile_hook = lambda h: setattr(m, "_hook", h)
            m.get_axon_ntff_profile_hook = lambda: m._hook
            sys.modules["antenv.axon_hooks"] = m
            antenv.axon_hooks = m
    except Exception:
        pass
_install_hooks()

import concourse.bass as bass
import concourse.tile as tile
from concourse import mybir, bacc
from concourse.bass_utils import run_bass_kernel_spmd

BF16 = mybir.dt.bfloat16
U16 = mybir.dt.uint16
F32 = mybir.dt.float32
FP8 = mybir.dt.float8e4
DRM = mybir.MatmulPerfMode.DoubleRow
AT = mybir.AluOpType
AF = mybir.ActivationFunctionType
bfloat16 = ml_dtypes.bfloat16
f8e4 = ml_dtypes.float8_e4m3

T, C, H, D, F = 2048, 1024, 16, 64, 4096
P, CH, Q4 = 128, 512, 256
NCO = C // P             # 8 feature tiles of the full model dim
HC, NHP = 8, 4           # heads per core, head-pairs per core
KC = HC * D              # 512 = per-core K/Q/V feature count
FH = F // 2              # 2048 = per-core fc hidden half
NF1 = FH // P            # 16
SA = 16.0
SW = 16.0
EXPSC = 0.125 / (SA * SW)
EPS = 1e-5
RG = [[0, 1], [2, 3], [4, 5], [6, 7]]

_cache = {}


def build_nc(apply_ln1w, apply_ln2w, add_bfc1, add_bfc2):
    nc = bacc.Bacc()
    xT_d = nc.declare_dram_parameter("xT", [P, NCO, T], BF16, isOutput=False)
    xqh_d = nc.declare_dram_parameter("xqh", [P, NCO, T], BF16, isOutput=False)
    wa_d = nc.declare_dram_parameter("wa_r", [P, NCO, 3 * KC], FP8, isOutput=False)
    wp_d = nc.declare_dram_parameter("wp_r", [P, NHP, C], FP8, isOutput=False)
    w1_d = nc.declare_dram_parameter("w1_r", [P, NCO, FH], BF16, isOutput=False)
    w2_d = nc.declare_dram_parameter("w2_r", [P, NF1, C], BF16, isOutput=False)
    dm_d = nc.declare_dram_parameter("dmask", [P, 4, CH], FP8, isOutput=False)
    ln1w_d = nc.declare_dram_parameter("ln1w_col", [P, NCO], F32, isOutput=False)
    ln2w_d = nc.declare_dram_parameter("ln2w_col", [P, NCO], F32, isOutput=False)
    b1_d = nc.declare_dram_parameter("bfc1_col", [P, NF1], F32, isOutput=False)
    b2_d = nc.declare_dram_parameter("bfc2_col", [P, NCO], F32, isOutput=False)
    out_d = nc.declare_dram_parameter("out", [C, 4 * Q4], BF16, isOutput=True)
    out_r = out_d.rearrange("(ct p) q -> p ct q", p=P)

    with tile.TileContext(nc) as tc:
        with tc.tile_pool(name="consts", bufs=1) as consts, \
             tc.tile_pool(name="persist", bufs=1) as persist, \
             tc.tile_pool(name="dram", bufs=1, space="DRAM") as dram:
            onesb_sb = consts.tile([P, P], BF16)
            nc.vector.memset(onesb_sb[:], 1.0)
            eps_sb = consts.tile([P, 1], F32)
            nc.vector.memset(eps_sb[:], EPS / (SA * SA))
            eps2_sb = consts.tile([P, 1], F32)
            nc.vector.memset(eps2_sb[:], EPS / (4 * SA * SA))
            dm_sb = consts.tile([P, 4, CH], FP8)
            nc.sync.dma_start(out=dm_sb[:], in_=dm_d[:])
            ln1w_sb = consts.tile([P, NCO], F32)
            if apply_ln1w:
                nc.sync.dma_start(out=ln1w_sb[:], in_=ln1w_d[:])
            ln2w_sb = consts.tile([P, NCO], F32)
            if apply_ln2w:
                nc.sync.dma_start(out=ln2w_sb[:], in_=ln2w_d[:])
            b1_sb = consts.tile([P, NF1], F32)
            if add_bfc1:
                nc.sync.dma_start(out=b1_sb[:], in_=b1_d[:])
            b2_sb = consts.tile([P, NCO], F32)
            if add_bfc2:
                nc.sync.dma_start(out=b2_sb[:], in_=b2_d[:])
            wp_sb = consts.tile([P, NHP, C], FP8)
            nc.sync.dma_start(out=wp_sb[:], in_=wp_d[:])

            kT = persist.tile([P, NHP, T], FP8)
            qT = persist.tile([P, NHP, T], FP8)
            # per-head width padded to 66 so the DoubleRow Ldweights step
            # (HC*66=528 bytes) is a multiple of 16; row D = softmax-denominator
            # ones, row D+1 = zero pad (never read)
            v_sb = persist.tile([P, T // P, HC, D + 2], FP8)
            nc.vector.memset(v_sb[:, :, :, D:D + 1], 1.0)
            nc.vector.memset(v_sb[:, :, :, D + 1:D + 2], 0.0)
            y_stage = persist.tile([P, NHP, CH], BF16)
            yT_my = persist.tile([P, NHP, CH], FP8)
            pp_st = persist.tile([P, NCO, CH], BF16)
            xos = pp_st  # temporal reuse: AR readback lands after pp_st is shipped
            xoh = persist.tile([P, NCO, CH], BF16)
            h2T = persist.tile([P, NCO, CH], BF16)
            a1 = persist.tile([P, NF1, CH], BF16)
            fst = persist.tile([P, NCO, CH], BF16)
            xqh = persist.tile([P, NCO, CH], BF16)

            den_d = dram.tile([HC, T], BF16)
            deni_d = dram.tile([HC, T], BF16)
            ar_in_d = [dram.tile([P, NCO, CH], BF16, name=f"ari{c}") for c in range(4)]
            ar_out_d = [dram.tile([P, NCO, CH], BF16, name=f"aro{c}") for c in range(4)]
            rs_in_d = [dram.tile([2 * P, NCO, Q4], BF16, name=f"rsi{c}") for c in range(4)]
            rs_out_d = [dram.tile([P, NCO, Q4], BF16, name=f"rso{c}") for c in range(4)]

            # ---------------- layernorm via stats-matmuls ----------------
            def ln_emit(p_ln, ps_ln, src_ap_fn, h_out_fn, w_sb, apply_w, ptag,
                        pbufs, eps_ap, h_dt_fp8):
                """src_ap_fn(co) -> [P, CH] bf16 AP; h_out_fn(co) -> [P, CH] out."""
                ps_mu = ps_ln.tile([P, CH], F32, tag=ptag, bufs=pbufs)
                ps_sq = ps_ln.tile([P, CH], F32, tag=ptag, bufs=pbufs)
                for co in range(NCO):
                    nc.tensor.matmul(ps_mu[:], onesb_sb[:], src_ap_fn(co),
                                     start=(co == 0), stop=(co == NCO - 1))
                    sq = p_ln.tile([P, CH], BF16, tag="sq", bufs=3)
                    nc.vector.tensor_mul(out=sq[:], in0=src_ap_fn(co), in1=src_ap_fn(co))
                    nc.tensor.matmul(ps_sq[:], onesb_sb[:], sq[:],
                                     start=(co == 0), stop=(co == NCO - 1))
                mu = p_ln.tile([P, CH], F32, tag="mu", bufs=1)
                nc.vector.tensor_scalar_mul(out=mu[:], in0=ps_mu[:], scalar1=1.0 / C)
                m2 = p_ln.tile([P, CH], F32, tag="m2", bufs=1)
                nc.vector.tensor_mul(out=m2[:], in0=mu[:], in1=mu[:])
                var = p_ln.tile([P, CH], F32, tag="var", bufs=1)
                nc.vector.scalar_tensor_tensor(
                    out=var[:], in0=ps_sq[:], scalar=1.0 / C, in1=m2[:],
                    op0=AT.mult, op1=AT.subtract)
                sd = p_ln.tile([P, CH], F32, tag="sd", bufs=1)
                nc.scalar.activation(out=sd[:], in_=var[:], func=AF.Sqrt,
                                     scale=1.0 / (SA * SA), bias=eps_ap)
                rstd = p_ln.tile([P, CH], F32, tag="rstd", bufs=1)
                nc.vector.reciprocal_approx_fast(out=rstd[:], in_=sd[:])
                for co in range(NCO):
                    dd = p_ln.tile([P, CH], F32, tag="dd", bufs=2)
                    nc.vector.tensor_sub(out=dd[:], in0=src_ap_fn(co), in1=mu[:])
                    if apply_w:
                        nc.vector.tensor_mul(out=dd[:], in0=dd[:], in1=rstd[:])
                        nc.vector.tensor_scalar_mul(
                            out=h_out_fn(co), in0=dd[:],
                            scalar1=w_sb[:, co:co + 1])
                    else:
                        nc.vector.tensor_mul(out=h_out_fn(co), in0=dd[:], in1=rstd[:])

            # ================= phase A: ln1 + K/Q/V ==========================
            with tc.tile_pool(name="p_h1", bufs=1) as p_h1, \
                 tc.tile_pool(name="p_wa", bufs=1) as p_wa, \
                 tc.tile_pool(name="p_ln", bufs=1) as p_ln, \
                 tc.tile_pool(name="p_xt", bufs=1) as p_xt, \
                 tc.tile_pool(name="ps_A", bufs=1, space="PSUM") as ps_A:
                h1T = p_h1.tile([P, NCO, T], FP8)
                wkq_sb = p_wa.tile([P, NCO, 2 * KC], FP8)
                nc.gpsimd.dma_start(out=wkq_sb[:], in_=wa_d[:, :, 0:2 * KC])
                wv_sb = p_wa.tile([P, NCO, KC], FP8)
                nc.gpsimd.dma_start(out=wv_sb[:], in_=wa_d[:, :, 2 * KC:3 * KC])

                def emit_chunk(c):
                    xt = p_xt.tile([P, NCO, CH], BF16, tag="xt", bufs=2, name=f"xt{c}")
                    nc.sync.dma_start(out=xt[:], in_=xT_d[:, :, c * CH:(c + 1) * CH])
                    ln_emit(p_ln, ps_A,
                            lambda co: xt[:, co, :],
                            lambda co: h1T[:, co, c * CH:(c + 1) * CH],
                            ln1w_sb, apply_ln1w, "lnA", 4, eps_sb[:], True)
                    # K, Q into kT/qT; V into v_sb
                    for which, dst in ((0, kT), (1, qT)):
                        for hp in range(NHP):
                            ps = ps_A.tile([P, CH], F32, tag="kqv", bufs=3,
                                           name=f"kq{c}_{which}_{hp}")
                            for j in range(4):
                                nc.tensor.matmul(
                                    ps[:],
                                    wkq_sb[:, 2 * j:2 * j + 2,
                                           which * KC + hp * P:which * KC + (hp + 1) * P],
                                    h1T[:, 2 * j:2 * j + 2, c * CH:(c + 1) * CH],
                                    start=(j == 0), stop=(j == 3), perf_mode=DRM)
                            nc.scalar.activation(
                                out=dst[:, hp, c * CH:(c + 1) * CH], in_=ps[:],
                                func=AF.Copy, scale=1.0 / SA)
                    for tt in range(4 * c, 4 * c + 4):
                        ps = ps_A.tile([P, KC], F32, tag="kqv", bufs=3, name=f"v{tt}")
                        for j in range(4):
                            nc.tensor.matmul(
                                ps[:],
                                h1T[:, 2 * j:2 * j + 2, tt * P:(tt + 1) * P],
                                wv_sb[:, 2 * j:2 * j + 2, :],
                                start=(j == 0), stop=(j == 3), perf_mode=DRM)
                        nc.scalar.activation(
                            out=v_sb[:, tt, :, 0:D],
                            in_=ps.rearrange("p (h d) -> p h d", d=D),
                            func=AF.Copy, scale=1.0 / SA)

                for c in range(4):
                    emit_chunk(c)

            # ---------------- attention emitter (shared) ---------------------
            def emit_attn(ci, hp, p_pt, p_yt, ps_st, ps_av, drain):
                    n_w = ci + 1
                    apvs = [ps_av.tile([D + 2, CH], F32, tag="av", bufs=2,
                                       name=f"av{ci}_{hp}_{i}") for i in range(2)]
                    for w in range(n_w):
                        pts = [p_pt.tile([P, 4, CH], FP8, tag="pt", bufs=4,
                                         name=f"pt{ci}_{hp}_{w}_{i}") for i in range(2)]
                        for kl in range(2):
                            sps = [ps_st.tile([P, 2 * CH], F32, tag="st", bufs=2,
                                              name=f"sp{ci}_{hp}_{w}_{kl}_{i}")
                                   for i in range(2)]
                            for k2 in range(2):
                                kt = 4 * w + 2 * kl + k2
                                for h2 in range(2):
                                    r0 = h2 * D
                                    nc.tensor.matmul(
                                        sps[h2][:, k2 * CH:(k2 + 1) * CH],
                                        kT[r0:r0 + D, hp, kt * P:(kt + 1) * P],
                                        qT[r0:r0 + D, hp, ci * CH:(ci + 1) * CH],
                                        start=True, stop=True)
                            for h2 in range(2):
                                nc.scalar.activation(
                                    out=pts[h2].rearrange("p k a -> p (k a)")[
                                        :, 2 * kl * CH:(2 * kl + 2) * CH],
                                    in_=sps[h2][:], func=AF.Exp, scale=EXPSC)
                            drain()
                        if w == n_w - 1:
                            for h2 in range(2):
                                pv = pts[h2][:].bitcast(U16)
                                nc.vector.tensor_tensor(
                                    out=pv, in0=pv, in1=dm_sb[:].bitcast(U16),
                                    op=AT.bitwise_and)
                        for t_in in range(2):
                            tt = 4 * w + 2 * t_in
                            for h2 in range(2):
                                nc.tensor.matmul(
                                    apvs[h2][:], v_sb[:, tt:tt + 2, 2 * hp + h2, :],
                                    pts[h2][:, 2 * t_in:2 * t_in + 2, :],
                                    start=(w == 0 and t_in == 0),
                                    stop=(w == n_w - 1 and t_in == 1),
                                    perf_mode=DRM)
                        drain()
                    nc.vector.tensor_copy(out=y_stage[0:D, hp, :], in_=apvs[0][0:D, :])
                    ytmp = p_yt.tile([D, CH], BF16, tag="ytmp", bufs=2)
                    nc.vector.tensor_copy(out=ytmp[:], in_=apvs[1][0:D, :])
                    nc.sync.dma_start(out=y_stage[D:P, hp, :], in_=ytmp[:])
                    dpr = p_yt.tile([P, 2, CH], BF16, tag="dpr", bufs=1)
                    for h2 in range(2):
                        nc.vector.tensor_copy(out=dpr[D:D + 1, h2, :],
                                              in_=apvs[h2][D:D + 1, :])
                    nc.gpsimd.dma_start(
                        out=den_d[2 * hp:2 * hp + 2, ci * CH:(ci + 1) * CH],
                        in_=dpr[D:D + 1, :, :])
                    drain()

            def no_drain():
                pass

            # ============== phase B: attention ci0-3 + MLP pipeline ===========
            with tc.tile_pool(name="p_w12", bufs=1) as p_w12, \
                 tc.tile_pool(name="p_dn", bufs=1) as p_dn, \
                 tc.tile_pool(name="p_pt", bufs=1) as p_pt, \
                 tc.tile_pool(name="p_yt", bufs=1) as p_yt, \
                 tc.tile_pool(name="ps_st", bufs=1, space="PSUM") as ps_st, \
                 tc.tile_pool(name="ps_av", bufs=1, space="PSUM") as ps_av, \
                 tc.tile_pool(name="ps_fill", bufs=1, space="PSUM") as ps_fill:
                w1_sb = p_w12.tile([P, NCO, FH], BF16)
                nc.gpsimd.dma_start(out=w1_sb[:], in_=w1_d[:])
                w2_sb = p_w12.tile([P, NF1, C], BF16)
                nc.gpsimd.dma_start(out=w2_sb[:], in_=w2_d[:])

                def emit_den_post(ci):
                    den_sb = p_dn.tile([HC, CH], BF16, tag="den", bufs=2)
                    nc.sync.dma_start(out=den_sb[:], in_=den_d[:, ci * CH:(ci + 1) * CH])
                    deni = p_dn.tile([HC, CH], F32, tag="deni", bufs=2)
                    nc.vector.reciprocal(out=deni[:], in_=den_sb[:])
                    denib = p_dn.tile([HC, CH], BF16, tag="denib", bufs=2)
                    nc.vector.tensor_copy(out=denib[:], in_=deni[:])
                    nc.sync.dma_start(out=deni_d[:, ci * CH:(ci + 1) * CH], in_=denib[:])
                    for hp in range(NHP):
                        dbc = p_dn.tile([P, CH], BF16, tag="dbc", bufs=2)
                        eng = nc.gpsimd if hp % 2 == 0 else nc.sync
                        for h2 in range(2):
                            row = deni_d[2 * hp + h2:2 * hp + h2 + 1,
                                         ci * CH:(ci + 1) * CH]
                            eng.dma_start(
                                out=dbc[h2 * D:(h2 + 1) * D, :],
                                in_=bass.AP(tensor=row.tensor, offset=row.offset,
                                            ap=[[0, D]] + [list(a) for a in row.ap]))
                        nc.vector.tensor_mul(out=yT_my[:, hp, :],
                                             in0=y_stage[:, hp, :], in1=dbc[:])

                def emit_proj_ar(ci, psp, psb=2):
                    for cf in range(NCO):
                        ps = psp.tile([P, CH], F32, tag="fill", bufs=psb)
                        for j in range(2):
                            nc.tensor.matmul(
                                ps[:], wp_sb[:, 2 * j:2 * j + 2, cf * P:(cf + 1) * P],
                                yT_my[:, 2 * j:2 * j + 2, :],
                                start=(j == 0), stop=(j == 1), perf_mode=DRM)
                        nc.vector.tensor_scalar_mul(out=pp_st[:, cf, :], in0=ps[:],
                                                    scalar1=1.0 / (SA * SW))
                    nc.sync.dma_start(out=ar_in_d[ci][:], in_=pp_st[:])
                    nc.gpsimd.collective_compute(
                        "AllReduce", AT.add, replica_groups=RG,
                        ins=[ar_in_d[ci][:]], outs=[ar_out_d[ci][:]])

                def emit_xo(ci):
                    nc.sync.dma_start(
                        out=xqh[:], in_=xqh_d[:, :, ci * CH:(ci + 1) * CH])
                    nc.sync.dma_start(out=xos[:], in_=ar_out_d[ci][:])
                    for co in range(NCO):
                        nc.vector.scalar_tensor_tensor(
                            out=xoh[:, co, :], in0=xos[:, co, :], scalar=0.5,
                            in1=xqh[:, co, :], op0=AT.mult, op1=AT.add)

                def emit_ln2(psp, psb=2):
                    ln_emit(p_dn, psp,
                            lambda co: xoh[:, co, :],
                            lambda co: h2T[:, co, :],
                            ln2w_sb, apply_ln2w, "fill", psb, eps2_sb[:], False)

                def emit_fc1(ft, psp, psb=2):
                    ps = psp.tile([P, CH], F32, tag="fill", bufs=psb)
                    for j in range(NCO):
                        nc.tensor.matmul(ps[:], w1_sb[:, j, ft * P:(ft + 1) * P],
                                         h2T[:, j, :], start=(j == 0),
                                         stop=(j == NCO - 1))
                    if add_bfc1:
                        nc.scalar.activation(
                            out=a1[:, ft, :], in_=ps[:], func=AF.Relu,
                            scale=1.0 / SW, bias=b1_sb[:, ft:ft + 1])
                    else:
                        nc.vector.tensor_scalar(
                            out=a1[:, ft, :], in0=ps[:], scalar1=1.0 / SW,
                            scalar2=0.0, op0=AT.mult, op1=AT.max)

                def emit_fc2(ct, psp, psb=2):
                    ps = psp.tile([P, CH], F32, tag="fill", bufs=psb)
                    for j in range(NF1):
                        nc.tensor.matmul(ps[:], w2_sb[:, j, ct * P:(ct + 1) * P],
                                         a1[:, j, :], start=(j == 0),
                                         stop=(j == NF1 - 1))
                    if add_bfc2:
                        nc.vector.tensor_scalar(
                            out=ps[:], in0=ps[:], scalar1=1.0 / (SA * SW),
                            scalar2=b2_sb[:, ct:ct + 1], op0=AT.mult, op1=AT.add)
                        nc.vector.tensor_scalar_mul(out=ps[:], in0=ps[:], scalar1=0.5)
                        nc.vector.tensor_add(out=fst[:, ct, :], in0=ps[:],
                                             in1=xoh[:, ct, :])
                    else:
                        nc.vector.scalar_tensor_tensor(
                            out=fst[:, ct, :], in0=ps[:], scalar=1.0 / (SA * SW),
                            in1=xoh[:, ct, :], op0=AT.mult, op1=AT.add)

                def emit_rs_out(ci):
                    for qq in range(2):
                        nc.sync.dma_start(
                            out=rs_in_d[ci][qq * P:(qq + 1) * P, :, :],
                            in_=fst[:, :, qq * Q4:(qq + 1) * Q4])
                    nc.gpsimd.collective_compute(
                        "ReduceScatter", AT.add, replica_groups=RG,
                        ins=[rs_in_d[ci][:]], outs=[rs_out_d[ci][:]])
                    nc.sync.dma_start(out=out_r[:, :, ci * Q4:(ci + 1) * Q4],
                                      in_=rs_out_d[ci][:])

                def mlp_items(ci):
                    items = [lambda: emit_xo(ci)]
                    items += [lambda: emit_ln2(ps_fill)]
                    items += [(lambda ft=ft: emit_fc1(ft, ps_fill)) for ft in range(NF1)]
                    items += [(lambda ct=ct: emit_fc2(ct, ps_fill)) for ct in range(NCO)]
                    items += [lambda: emit_rs_out(ci)]
                    return items

                def make_drain(queue, n_calls):
                    state = {'calls': 0, 'done': 0}
                    total = len(queue)

                    def drain():
                        state['calls'] += 1
                        want = (total * state['calls']) // n_calls
                        while state['done'] < min(want, total):
                            queue[state['done']]()
                            state['done'] += 1
                    return drain

                for hp in range(NHP):
                    emit_attn(0, hp, p_pt, p_yt, ps_st, ps_av, no_drain)
                emit_den_post(0)
                emit_proj_ar(0, ps_fill)
                for ci in range(1, 4):
                    fillq = mlp_items(ci - 1)
                    # drain calls per (hp, window): 2 ktp + 1 av + 1 tail
                    n_calls = NHP * ((ci + 1) * 3 + 1)
                    drain = make_drain(fillq, n_calls)
                    for hp in range(NHP):
                        emit_attn(ci, hp, p_pt, p_yt, ps_st, ps_av, drain)
                    emit_den_post(ci)
                    emit_proj_ar(ci, ps_fill)
                # tail: MLP(3)
                for it in mlp_items(3):
                    it()

    nc.compile()
    return nc


def _prep(x, ln1_w, w_attn, w_proj, ln2_w, w_fc1, b_fc1, w_fc2, b_fc2):
    x = np.asarray(x, np.float32)
    B = x.shape[0]
    apply_ln1w = not np.allclose(ln1_w, 1.0)
    apply_ln2w = not np.allclose(ln2_w, 1.0)
    add_bfc1 = not np.allclose(b_fc1, 0.0)
    add_bfc2 = not np.allclose(b_fc2, 0.0)
    key = (apply_ln1w, apply_ln2w, add_bfc1, add_bfc2)
    if key not in _cache:
        _cache[key] = build_nc(*key)
    nc = _cache[key]

    def r3(w, n):  # [rows, n] -> [p, rows//P, n] partition-major
        w = np.asarray(w, np.float32)
        return w.reshape(w.shape[0] // P, P, n).transpose(1, 0, 2)

    def c8(a, s):
        return np.ascontiguousarray(a * s).astype(f8e4)

    def cb(a, s):
        return np.ascontiguousarray(a * s).astype(bfloat16)

    # shared diag byte mask [P, 4, CH]: valid iff col >= 128*j + p
    pidx = np.arange(P)[:, None]
    col = np.arange(CH)[None, :]
    dm = np.stack([(col >= (P * j + pidx)) for j in range(4)], 1)
    dm = (dm.astype(np.uint8) * np.uint8(0xFF)).view(f8e4).reshape(P, 4, CH)

    w_attn = np.asarray(w_attn, np.float32)
    per_par = []
    for p in range(2):
        qs = w_attn[:, p * KC:(p + 1) * KC]
        ks = w_attn[:, C + p * KC:C + (p + 1) * KC]
        vs = w_attn[:, 2 * C + p * KC:2 * C + (p + 1) * KC]
        wa = np.concatenate([ks, qs, vs], axis=1)       # [K|Q|V]
        wp_rows = np.asarray(w_proj, np.float32)[p * KC:(p + 1) * KC]
        w1h = np.asarray(w_fc1, np.float32)[:, p * FH:(p + 1) * FH]
        w2h = np.asarray(w_fc2, np.float32)[p * FH:(p + 1) * FH, :]
        b1h = np.asarray(b_fc1, np.float32)[p * FH:(p + 1) * FH]
        per_par.append(dict(
            wa_r=c8(r3(wa, 3 * KC), SW),
            wp_r=c8(wp_rows.reshape(NHP, P, C).transpose(1, 0, 2), SW),
            w1_r=cb(r3(w1h, FH), SW),
            w2_r=cb(w2h.reshape(NF1, P, C).transpose(1, 0, 2), SW),
            bfc1_col=np.ascontiguousarray(b1h.reshape(NF1, P).T * SA),
            dmask=dm,
            ln1w_col=np.ascontiguousarray(
                np.asarray(ln1_w, np.float32).reshape(NCO, P).T),
            ln2w_col=np.ascontiguousarray(
                np.asarray(ln2_w, np.float32).reshape(NCO, P).T),
            bfc2_col=np.ascontiguousarray(
                np.asarray(b_fc2, np.float32).reshape(NCO, P).T),
        ))

    in_maps = []
    for core in range(2 * B):
        s, p = core // 2, core % 2
        xs = x[s]                                     # [T, C]
        xsT = xs.T                                    # [C, T]
        xT = np.ascontiguousarray(
            xsT.reshape(NCO, P, T).transpose(1, 0, 2)).astype(bfloat16)
        xqh = np.ascontiguousarray(
            (xsT * 0.5).reshape(NCO, P, T).transpose(1, 0, 2)).astype(bfloat16)
        in_maps.append(dict(per_par[p], xT=xT, xqh=xqh))
    return nc, in_maps, x


def kernel(x, ln1_w, w_attn, w_proj, ln2_w, w_fc1, b_fc1, w_fc2, b_fc2):
    nc, in_maps, x = _prep(x, ln1_w, w_attn, w_proj, ln2_w,
                           w_fc1, b_fc1, w_fc2, b_fc2)
    B = x.shape[0]
    res = run_bass_kernel_spmd(nc, in_maps, list(range(2 * B)))
    out = np.empty_like(x)
    for core in range(2 * B):
        s, p = core // 2, core % 2
        r = np.asarray(res.results[core]["out"], np.float32)  # [C, 4*Q4]
        for ci in range(4):
            toks = 512 * ci + 256 * p
            out[s, toks:toks + Q4] = r[:, ci * Q4:(ci + 1) * Q4].T
    return out


# revision 25
# speedup vs baseline: 1.0112x; 1.0112x over previous
"""Trainium2 Bass kernel v4 for the dense transformer block (B=4,T=2048,C=1024,H=16,F=4096).

Sharding (head-parallel attention + Megatron MLP across core pairs):
- core 2s+p owns sequence s; parity p owns heads 8p..8p+8 and computes
  K/Q/V + scores/softmax/AV for those heads over ALL T tokens. Every core
  sees the same symmetric causal structure: chunk ci (512 q) needs exactly
  4(ci+1) k-tiles, the last 4 straddling the diagonal -> one shared
  [P,4,CH] byte mask, zero wasted score/exp work, no parity masks.
- proj is row-split by heads: each core computes a partial [C, 512] from its
  own y; partials are summed with a pairwise DRAM AllReduce. Both cores then
  run LN2 (duplicated, on (x + proj)/2 with eps/4) and a Megatron F-split
  fc1/fc2 (each core holds HALF of w_fc1/w_fc2 resident in SBUF - no weight
  streaming). fc2 partials + xo/2 go through a pairwise ReduceScatter(add)
  which lands each core's q-quarter of the final output, DMA'd DRAM->DRAM
  to the output tensor.
- Scores/exp masking via uint16 bitwise-AND on byte masks (DVE 2x mode).
- MLP(ci) is emitted as PE filler micro-blocks between attention score
  groups of ci+1; attention ci0 is interleaved into the LN1/QKV phase.
"""
import sys, types
import numpy as np
import ml_dtypes


def _install_hooks():
    try:
        import antenv
        if "antenv.axon_hooks" not in sys.modules:
            m = types.ModuleType("antenv.axon_hooks")
            m._hook = None
            m.set_axon_ntff_profile_hook = lambda h: setattr(m, "_hook", h)
            m.get_axon_ntff_profile_hook = lambda: m._hook
            sys.modules["antenv.axon_hooks"] = m
            antenv.axon_hooks = m
    except Exception:
        pass
_install_hooks()

import concourse.bass as bass
import concourse.tile as tile
from concourse import mybir, bacc
from concourse.bass_utils import run_bass_kernel_spmd

BF16 = mybir.dt.bfloat16
U16 = mybir.dt.uint16
F32 = mybir.dt.float32
FP8 = mybir.dt.float8e4
DRM = mybir.MatmulPerfMode.DoubleRow
AT = mybir.AluOpType
AF = mybir.ActivationFunctionType
bfloat16 = ml_dtypes.bfloat16
f8e4 = ml_dtypes.float8_e4m3

T, C, H, D, F = 2048, 1024, 16, 64, 4096
P, CH, Q4 = 128, 512, 256
NCO = C // P             # 8 feature tiles of the full model dim
HC, NHP = 8, 4           # heads per core, head-pairs per core
KC = HC * D              # 512 = per-core K/Q/V feature count
FH = F // 2              # 2048 = per-core fc hidden half
NF1 = FH // P            # 16
SA = 16.0
SW = 16.0
EXPSC = 0.125 / (SA * SW)
EPS = 1e-5
RG = [[0, 1], [2, 3], [4, 5], [6, 7]]

_cache = {}


def build_nc(apply_ln1w, apply_ln2w, add_bfc1, add_bfc2):
    nc = bacc.Bacc()
    xT_d = nc.declare_dram_parameter("xT", [P, NCO, T], BF16, isOutput=False)
    xqh_d = nc.declare_dram_parameter("xqh", [P, NCO, T], BF16, isOutput=False)
    wa_d = nc.declare_dram_parameter("wa_r", [P, NCO, 3 * KC], FP8, isOutput=False)
    wp_d = nc.declare_dram_parameter("wp_r", [P, NHP, C], FP8, isOutput=False)
    w1_d = nc.declare_dram_parameter("w1_r", [P, NCO, FH], BF16, isOutput=False)
    w2_d = nc.declare_dram_parameter("w2_r", [P, NF1, C], BF16, isOutput=False)
    dm_d = nc.declare_dram_parameter("dmask", [P, 4, CH], FP8, isOutput=False)
    ln1w_d = nc.declare_dram_parameter("ln1w_col", [P, NCO], F32, isOutput=False)
    ln2w_d = nc.declare_dram_parameter("ln2w_col", [P, NCO], F32, isOutput=False)
    b1_d = nc.declare_dram_parameter("bfc1_col", [P, NF1], F32, isOutput=False)
    b2_d = nc.declare_dram_parameter("bfc2_col", [P, NCO], F32, isOutput=False)
    out_d = nc.declare_dram_parameter("out", [C, 4 * Q4], BF16, isOutput=True)
    out_r = out_d.rearrange("(ct p) q -> p ct q", p=P)

    with tile.TileContext(nc) as tc:
        with tc.tile_pool(name="consts", bufs=1) as consts, \
             tc.tile_pool(name="persist", bufs=1) as persist, \
             tc.tile_pool(name="dram", bufs=1, space="DRAM") as dram:
            onesb_sb = consts.tile([P, P], BF16)
            nc.vector.memset(onesb_sb[:], 1.0)
            eps_sb = consts.tile([P, 1], F32)
            nc.vector.memset(eps_sb[:], EPS / (SA * SA))
            eps2_sb = consts.tile([P, 1], F32)
            nc.vector.memset(eps2_sb[:], EPS / (4 * SA * SA))
            dm_sb = consts.tile([P, 4, CH], FP8)
            nc.sync.dma_start(out=dm_sb[:], in_=dm_d[:])
            ln1w_sb = consts.tile([P, NCO], F32)
            if apply_ln1w:
                nc.sync.dma_start(out=ln1w_sb[:], in_=ln1w_d[:])
            ln2w_sb = consts.tile([P, NCO], F32)
            if apply_ln2w:
                nc.sync.dma_start(out=ln2w_sb[:], in_=ln2w_d[:])
            b1_sb = consts.tile([P, NF1], F32)
            if add_bfc1:
                nc.sync.dma_start(out=b1_sb[:], in_=b1_d[:])
            b2_sb = consts.tile([P, NCO], F32)
            if add_bfc2:
                nc.sync.dma_start(out=b2_sb[:], in_=b2_d[:])
            wp_sb = consts.tile([P, NHP, C], FP8)
            nc.sync.dma_start(out=wp_sb[:], in_=wp_d[:])

            kT = persist.tile([P, NHP, T], FP8)
            qT = persist.tile([P, NHP, T], FP8)
            # per-head width padded to 66 so the DoubleRow Ldweights step
            # (HC*66=528 bytes) is a multiple of 16; row D = softmax-denominator
            # ones, row D+1 = zero pad (never read)
            v_sb = persist.tile([P, T // P, HC, D + 2], FP8)
            nc.vector.memset(v_sb[:, :, :, D:D + 1], 1.0)
            nc.vector.memset(v_sb[:, :, :, D + 1:D + 2], 0.0)
            y_stage = persist.tile([P, NHP, CH], BF16)
            yT_my = persist.tile([P, NHP, CH], FP8)
            pp_st = persist.tile([P, NCO, CH], BF16)
            xos = pp_st  # temporal reuse: AR readback lands after pp_st is shipped
            xoh = persist.tile([P, NCO, CH], BF16)
            h2T = persist.tile([P, NCO, CH], BF16)
            a1 = persist.tile([P, NF1, CH], BF16)
            fst = persist.tile([P, NCO, CH], BF16)
            xqh = persist.tile([P, NCO, CH], BF16)

            den_d = dram.tile([HC, T], BF16)
            deni_d = dram.tile([HC, T], BF16)
            ar_in_d = [dram.tile([P, NCO, CH], BF16, name=f"ari{c}") for c in range(4)]
            ar_out_d = [dram.tile([P, NCO, CH], BF16, name=f"aro{c}") for c in range(4)]
            rs_in_d = [dram.tile([2 * P, NCO, Q4], BF16, name=f"rsi{c}") for c in range(4)]
            rs_out_d = [dram.tile([P, NCO, Q4], BF16, name=f"rso{c}") for c in range(4)]

            # ---------------- layernorm via stats-matmuls ----------------
            def ln_stats(p_ln, ps_ln, src_ap_fn, ptag, pbufs):
                ps_mu = ps_ln.tile([P, CH], F32, tag=ptag, bufs=pbufs)
                ps_sq = ps_ln.tile([P, CH], F32, tag=ptag, bufs=pbufs)
                for co in range(NCO):
                    nc.tensor.matmul(ps_mu[:], onesb_sb[:], src_ap_fn(co),
                                     start=(co == 0), stop=(co == NCO - 1))
                    sq = p_ln.tile([P, CH], BF16, tag="sq", bufs=3)
                    nc.vector.tensor_mul(out=sq[:], in0=src_ap_fn(co), in1=src_ap_fn(co))
                    nc.tensor.matmul(ps_sq[:], onesb_sb[:], sq[:],
                                     start=(co == 0), stop=(co == NCO - 1))
                return ps_mu, ps_sq

            def ln_finish(p_ln, stats, src_ap_fn, h_out_fn, w_sb, apply_w, eps_ap):
                ps_mu, ps_sq = stats
                mu = p_ln.tile([P, CH], F32, tag="mu", bufs=2)
                nc.vector.tensor_scalar_mul(out=mu[:], in0=ps_mu[:], scalar1=1.0 / C)
                m2 = p_ln.tile([P, CH], F32, tag="m2", bufs=2)
                nc.vector.tensor_mul(out=m2[:], in0=mu[:], in1=mu[:])
                var = p_ln.tile([P, CH], F32, tag="var", bufs=2)
                nc.vector.scalar_tensor_tensor(
                    out=var[:], in0=ps_sq[:], scalar=1.0 / C, in1=m2[:],
                    op0=AT.mult, op1=AT.subtract)
                sd = p_ln.tile([P, CH], F32, tag="sd", bufs=2)
                nc.scalar.activation(out=sd[:], in_=var[:], func=AF.Sqrt,
                                     scale=1.0 / (SA * SA), bias=eps_ap)
                rstd = p_ln.tile([P, CH], F32, tag="rstd", bufs=2)
                nc.vector.reciprocal_approx_fast(out=rstd[:], in_=sd[:])
                for co in range(NCO):
                    dd = p_ln.tile([P, CH], F32, tag="dd", bufs=2)
                    nc.vector.tensor_sub(out=dd[:], in0=src_ap_fn(co), in1=mu[:])
                    if apply_w:
                        nc.vector.tensor_mul(out=dd[:], in0=dd[:], in1=rstd[:])
                        nc.vector.tensor_scalar_mul(
                            out=h_out_fn(co), in0=dd[:],
                            scalar1=w_sb[:, co:co + 1])
                    else:
                        nc.vector.tensor_mul(out=h_out_fn(co), in0=dd[:], in1=rstd[:])

            def ln_emit(p_ln, ps_ln, src_ap_fn, h_out_fn, w_sb, apply_w, ptag,
                        pbufs, eps_ap, h_dt_fp8):
                stats = ln_stats(p_ln, ps_ln, src_ap_fn, ptag, pbufs)
                ln_finish(p_ln, stats, src_ap_fn, h_out_fn, w_sb, apply_w, eps_ap)

            # ================= phase A: ln1 + K/Q/V ==========================
            with tc.tile_pool(name="p_h1", bufs=1) as p_h1, \
                 tc.tile_pool(name="p_wa", bufs=1) as p_wa, \
                 tc.tile_pool(name="p_ln", bufs=1) as p_ln, \
                 tc.tile_pool(name="p_xt", bufs=1) as p_xt, \
                 tc.tile_pool(name="ps_A", bufs=1, space="PSUM") as ps_A:
                h1T = p_h1.tile([P, NCO, T], FP8)
                wkq_sb = p_wa.tile([P, NCO, 2 * KC], FP8)
                nc.gpsimd.dma_start(out=wkq_sb[:], in_=wa_d[:, :, 0:2 * KC])
                wv_sb = p_wa.tile([P, NCO, KC], FP8)
                nc.gpsimd.dma_start(out=wv_sb[:], in_=wa_d[:, :, 2 * KC:3 * KC])

                def emit_stats_A(c):
                    xt = p_xt.tile([P, NCO, CH], BF16, tag="xt", bufs=2, name=f"xt{c}")
                    nc.sync.dma_start(out=xt[:], in_=xT_d[:, :, c * CH:(c + 1) * CH])
                    st = ln_stats(p_ln, ps_A, lambda co: xt[:, co, :], "lnA", 4)
                    return xt, st

                def emit_chunk(c, xt, st):
                    ln_finish(p_ln, st,
                              lambda co: xt[:, co, :],
                              lambda co: h1T[:, co, c * CH:(c + 1) * CH],
                              ln1w_sb, apply_ln1w, eps_sb[:])
                    # K, Q into kT/qT; V into v_sb
                    for which, dst in ((0, kT), (1, qT)):
                        for hp in range(NHP):
                            ps = ps_A.tile([P, CH], F32, tag="kqv", bufs=3,
                                           name=f"kq{c}_{which}_{hp}")
                            for j in range(4):
                                nc.tensor.matmul(
                                    ps[:],
                                    wkq_sb[:, 2 * j:2 * j + 2,
                                           which * KC + hp * P:which * KC + (hp + 1) * P],
                                    h1T[:, 2 * j:2 * j + 2, c * CH:(c + 1) * CH],
                                    start=(j == 0), stop=(j == 3), perf_mode=DRM)
                            nc.scalar.activation(
                                out=dst[:, hp, c * CH:(c + 1) * CH], in_=ps[:],
                                func=AF.Copy, scale=1.0 / SA)
                    for tt in range(4 * c, 4 * c + 4):
                        ps = ps_A.tile([P, KC], F32, tag="kqv", bufs=3, name=f"v{tt}")
                        for j in range(4):
                            nc.tensor.matmul(
                                ps[:],
                                h1T[:, 2 * j:2 * j + 2, tt * P:(tt + 1) * P],
                                wv_sb[:, 2 * j:2 * j + 2, :],
                                start=(j == 0), stop=(j == 3), perf_mode=DRM)
                        nc.scalar.activation(
                            out=v_sb[:, tt, :, 0:D],
                            in_=ps.rearrange("p (h d) -> p h d", d=D),
                            func=AF.Copy, scale=1.0 / SA)

                pend = emit_stats_A(0)
                for c in range(4):
                    cur, pend = pend, (emit_stats_A(c + 1) if c + 1 < 4 else None)
                    emit_chunk(c, *cur)

            # ---------------- attention emitter (shared) ---------------------
            def emit_attn(ci, hp, p_pt, p_yt, ps_st, ps_av, drain):
                    n_w = ci + 1
                    apvs = [ps_av.tile([D + 2, CH], F32, tag="av", bufs=2,
                                       name=f"av{ci}_{hp}_{i}") for i in range(2)]
                    for w in range(n_w):
                        pts = [p_pt.tile([P, 4, CH], FP8, tag="pt", bufs=4,
                                         name=f"pt{ci}_{hp}_{w}_{i}") for i in range(2)]
                        for kl in range(2):
                            sps = [ps_st.tile([P, 2 * CH], F32, tag="st", bufs=2,
                                              name=f"sp{ci}_{hp}_{w}_{kl}_{i}")
                                   for i in range(2)]
                            for k2 in range(2):
                                kt = 4 * w + 2 * kl + k2
                                for h2 in range(2):
                                    r0 = h2 * D
                                    nc.tensor.matmul(
                                        sps[h2][:, k2 * CH:(k2 + 1) * CH],
                                        kT[r0:r0 + D, hp, kt * P:(kt + 1) * P],
                                        qT[r0:r0 + D, hp, ci * CH:(ci + 1) * CH],
                                        start=True, stop=True)
                            for h2 in range(2):
                                nc.scalar.activation(
                                    out=pts[h2].rearrange("p k a -> p (k a)")[
                                        :, 2 * kl * CH:(2 * kl + 2) * CH],
                                    in_=sps[h2][:], func=AF.Exp, scale=EXPSC)
                            drain()
                        if w == n_w - 1:
                            for h2 in range(2):
                                pv = pts[h2][:].bitcast(U16)
                                nc.vector.tensor_tensor(
                                    out=pv, in0=pv, in1=dm_sb[:].bitcast(U16),
                                    op=AT.bitwise_and)
                        for t_in in range(2):
                            tt = 4 * w + 2 * t_in
                            for h2 in range(2):
                                nc.tensor.matmul(
                                    apvs[h2][:], v_sb[:, tt:tt + 2, 2 * hp + h2, :],
                                    pts[h2][:, 2 * t_in:2 * t_in + 2, :],
                                    start=(w == 0 and t_in == 0),
                                    stop=(w == n_w - 1 and t_in == 1),
                                    perf_mode=DRM)
                        drain()
                    nc.vector.tensor_copy(out=y_stage[0:D, hp, :], in_=apvs[0][0:D, :])
                    ytmp = p_yt.tile([D, CH], BF16, tag="ytmp", bufs=2)
                    nc.vector.tensor_copy(out=ytmp[:], in_=apvs[1][0:D, :])
                    nc.sync.dma_start(out=y_stage[D:P, hp, :], in_=ytmp[:])
                    dpr = p_yt.tile([P, 2, CH], BF16, tag="dpr", bufs=1)
                    for h2 in range(2):
                        nc.vector.tensor_copy(out=dpr[D:D + 1, h2, :],
                                              in_=apvs[h2][D:D + 1, :])
                    nc.gpsimd.dma_start(
                        out=den_d[2 * hp:2 * hp + 2, ci * CH:(ci + 1) * CH],
                        in_=dpr[D:D + 1, :, :])
                    drain()

            def no_drain():
                pass

            # ============== phase B: attention ci0-3 + MLP pipeline ===========
            with tc.tile_pool(name="p_w12", bufs=1) as p_w12, \
                 tc.tile_pool(name="p_dn", bufs=1) as p_dn, \
                 tc.tile_pool(name="p_pt", bufs=1) as p_pt, \
                 tc.tile_pool(name="p_yt", bufs=1) as p_yt, \
                 tc.tile_pool(name="ps_st", bufs=1, space="PSUM") as ps_st, \
                 tc.tile_pool(name="ps_av", bufs=1, space="PSUM") as ps_av, \
                 tc.tile_pool(name="ps_fill", bufs=1, space="PSUM") as ps_fill:
                w1_sb = p_w12.tile([P, NCO, FH], BF16)
                nc.gpsimd.dma_start(out=w1_sb[:], in_=w1_d[:])
                w2_sb = p_w12.tile([P, NF1, C], BF16)
                nc.gpsimd.dma_start(out=w2_sb[:], in_=w2_d[:])

                def emit_den_hp(ci, hp):
                    # normalize this head-pair's y as soon as its AV is done
                    den_sb = p_dn.tile([2, CH], BF16, tag="den", bufs=2)
                    nc.sync.dma_start(
                        out=den_sb[:],
                        in_=den_d[2 * hp:2 * hp + 2, ci * CH:(ci + 1) * CH])
                    deni = p_dn.tile([2, CH], F32, tag="deni", bufs=2)
                    nc.vector.reciprocal(out=deni[:], in_=den_sb[:])
                    denib = p_dn.tile([2, CH], BF16, tag="denib", bufs=2)
                    nc.vector.tensor_copy(out=denib[:], in_=deni[:])
                    nc.sync.dma_start(
                        out=deni_d[2 * hp:2 * hp + 2, ci * CH:(ci + 1) * CH],
                        in_=denib[:])
                    dbc = p_dn.tile([P, CH], BF16, tag="dbc", bufs=2)
                    eng = nc.gpsimd if hp % 2 == 0 else nc.sync
                    for h2 in range(2):
                        row = deni_d[2 * hp + h2:2 * hp + h2 + 1,
                                     ci * CH:(ci + 1) * CH]
                        eng.dma_start(
                            out=dbc[h2 * D:(h2 + 1) * D, :],
                            in_=bass.AP(tensor=row.tensor, offset=row.offset,
                                        ap=[[0, D]] + [list(a) for a in row.ap]))
                    nc.vector.tensor_mul(out=yT_my[:, hp, :],
                                         in0=y_stage[:, hp, :], in1=dbc[:])

                def emit_proj_ar(ci, psp, psb=2):
                    for cf in range(NCO):
                        ps = psp.tile([P, CH], F32, tag="fill", bufs=psb)
                        for j in range(2):
                            nc.tensor.matmul(
                                ps[:], wp_sb[:, 2 * j:2 * j + 2, cf * P:(cf + 1) * P],
                                yT_my[:, 2 * j:2 * j + 2, :],
                                start=(j == 0), stop=(j == 1), perf_mode=DRM)
                        nc.vector.tensor_scalar_mul(out=pp_st[:, cf, :], in0=ps[:],
                                                    scalar1=1.0 / (SA * SW))
                    nc.sync.dma_start(out=ar_in_d[ci][:], in_=pp_st[:])
                    nc.gpsimd.collective_compute(
                        "AllReduce", AT.add, replica_groups=RG,
                        ins=[ar_in_d[ci][:]], outs=[ar_out_d[ci][:]])

                def emit_xo(ci):
                    nc.sync.dma_start(
                        out=xqh[:], in_=xqh_d[:, :, ci * CH:(ci + 1) * CH])
                    nc.sync.dma_start(out=xos[:], in_=ar_out_d[ci][:])
                    for co in range(NCO):
                        nc.vector.scalar_tensor_tensor(
                            out=xoh[:, co, :], in0=xos[:, co, :], scalar=0.5,
                            in1=xqh[:, co, :], op0=AT.mult, op1=AT.add)

                def emit_ln2(psp, psb=2):
                    ln_emit(p_dn, psp,
                            lambda co: xoh[:, co, :],
                            lambda co: h2T[:, co, :],
                            ln2w_sb, apply_ln2w, "fill", psb, eps2_sb[:], False)

                def emit_fc1(ft, psp, psb=2):
                    ps = psp.tile([P, CH], F32, tag="fill", bufs=psb)
                    for j in range(NCO):
                        nc.tensor.matmul(ps[:], w1_sb[:, j, ft * P:(ft + 1) * P],
                                         h2T[:, j, :], start=(j == 0),
                                         stop=(j == NCO - 1))
                    if add_bfc1:
                        nc.scalar.activation(
                            out=a1[:, ft, :], in_=ps[:], func=AF.Relu,
                            scale=1.0 / SW, bias=b1_sb[:, ft:ft + 1])
                    else:
                        nc.vector.tensor_scalar(
                            out=a1[:, ft, :], in0=ps[:], scalar1=1.0 / SW,
                            scalar2=0.0, op0=AT.mult, op1=AT.max)

                def emit_fc2(ct, psp, psb=2):
                    ps = psp.tile([P, CH], F32, tag="fill", bufs=psb)
                    for j in range(NF1):
                        nc.tensor.matmul(ps[:], w2_sb[:, j, ct * P:(ct + 1) * P],
                                         a1[:, j, :], start=(j == 0),
                                         stop=(j == NF1 - 1))
                    if add_bfc2:
                        nc.vector.tensor_scalar(
                            out=ps[:], in0=ps[:], scalar1=1.0 / (SA * SW),
                            scalar2=b2_sb[:, ct:ct + 1], op0=AT.mult, op1=AT.add)
                        nc.vector.tensor_scalar_mul(out=ps[:], in0=ps[:], scalar1=0.5)
                        nc.vector.tensor_add(out=fst[:, ct, :], in0=ps[:],
                                             in1=xoh[:, ct, :])
                    else:
                        nc.vector.scalar_tensor_tensor(
                            out=fst[:, ct, :], in0=ps[:], scalar=1.0 / (SA * SW),
                            in1=xoh[:, ct, :], op0=AT.mult, op1=AT.add)

                def emit_rs_out(ci):
                    for qq in range(2):
                        nc.sync.dma_start(
                            out=rs_in_d[ci][qq * P:(qq + 1) * P, :, :],
                            in_=fst[:, :, qq * Q4:(qq + 1) * Q4])
                    nc.gpsimd.collective_compute(
                        "ReduceScatter", AT.add, replica_groups=RG,
                        ins=[rs_in_d[ci][:]], outs=[rs_out_d[ci][:]])
                    nc.sync.dma_start(out=out_r[:, :, ci * Q4:(ci + 1) * Q4],
                                      in_=rs_out_d[ci][:])

                def mlp_items(ci):
                    items = [lambda: emit_xo(ci)]
                    items += [lambda: emit_ln2(ps_fill)]
                    items += [(lambda ft=ft: emit_fc1(ft, ps_fill)) for ft in range(NF1)]
                    items += [(lambda ct=ct: emit_fc2(ct, ps_fill)) for ct in range(NCO)]
                    items += [lambda: emit_rs_out(ci)]
                    return items

                def make_drain(queue, n_calls, skip=0):
                    state = {'calls': 0, 'done': 0}
                    total = len(queue)

                    def drain():
                        state['calls'] += 1
                        eff = state['calls'] - skip
                        if eff <= 0:
                            return
                        want = (total * eff) // max(1, n_calls - skip)
                        while state['done'] < min(want, total):
                            queue[state['done']]()
                            state['done'] += 1
                    return drain

                for hp in range(NHP):
                    emit_attn(0, hp, p_pt, p_yt, ps_st, ps_av, no_drain)
                    emit_den_hp(0, hp)
                emit_proj_ar(0, ps_fill)
                for ci in range(1, 4):
                    fillq = mlp_items(ci - 1)
                    # drain calls per (hp, window): 2 ktp + 1 av + 1 tail.
                    # skip the first ~2 windows' worth so fillers never enter
                    # the in-order PE queue before their AllReduce has landed
                    n_calls = NHP * ((ci + 1) * 3 + 1)
                    skip = 2 * 3 + 1
                    drain = make_drain(fillq, n_calls, skip=skip)
                    for hp in range(NHP):
                        emit_attn(ci, hp, p_pt, p_yt, ps_st, ps_av, drain)
                        emit_den_hp(ci, hp)
                    emit_proj_ar(ci, ps_fill)
                # tail: MLP(3)
                for it in mlp_items(3):
                    it()

    nc.compile()
    return nc


def _prep(x, ln1_w, w_attn, w_proj, ln2_w, w_fc1, b_fc1, w_fc2, b_fc2):
    x = np.asarray(x, np.float32)
    B = x.shape[0]
    apply_ln1w = not np.allclose(ln1_w, 1.0)
    apply_ln2w = not np.allclose(ln2_w, 1.0)
    add_bfc1 = not np.allclose(b_fc1, 0.0)
    add_bfc2 = not np.allclose(b_fc2, 0.0)
    key = (apply_ln1w, apply_ln2w, add_bfc1, add_bfc2)
    if key not in _cache:
        _cache[key] = build_nc(*key)
    nc = _cache[key]

    def r3(w, n):  # [rows, n] -> [p, rows//P, n] partition-major
        w = np.asarray(w, np.float32)
        return w.reshape(w.shape[0] // P, P, n).transpose(1, 0, 2)

    def c8(a, s):
        return np.ascontiguousarray(a * s).astype(f8e4)

    def cb(a, s):
        return np.ascontiguousarray(a * s).astype(bfloat16)

    # shared diag byte mask [P, 4, CH]: valid iff col >= 128*j + p
    pidx = np.arange(P)[:, None]
    col = np.arange(CH)[None, :]
    dm = np.stack([(col >= (P * j + pidx)) for j in range(4)], 1)
    dm = (dm.astype(np.uint8) * np.uint8(0xFF)).view(f8e4).reshape(P, 4, CH)

    w_attn = np.asarray(w_attn, np.float32)
    per_par = []
    for p in range(2):
        qs = w_attn[:, p * KC:(p + 1) * KC]
        ks = w_attn[:, C + p * KC:C + (p + 1) * KC]
        vs = w_attn[:, 2 * C + p * KC:2 * C + (p + 1) * KC]
        wa = np.concatenate([ks, qs, vs], axis=1)       # [K|Q|V]
        wp_rows = np.asarray(w_proj, np.float32)[p * KC:(p + 1) * KC]
        w1h = np.asarray(w_fc1, np.float32)[:, p * FH:(p + 1) * FH]
        w2h = np.asarray(w_fc2, np.float32)[p * FH:(p + 1) * FH, :]
        b1h = np.asarray(b_fc1, np.float32)[p * FH:(p + 1) * FH]
        per_par.append(dict(
            wa_r=c8(r3(wa, 3 * KC), SW),
            wp_r=c8(wp_rows.reshape(NHP, P, C).transpose(1, 0, 2), SW),
            w1_r=cb(r3(w1h, FH), SW),
            w2_r=cb(w2h.reshape(NF1, P, C).transpose(1, 0, 2), SW),
            bfc1_col=np.ascontiguousarray(b1h.reshape(NF1, P).T * SA),
            dmask=dm,
            ln1w_col=np.ascontiguousarray(
                np.asarray(ln1_w, np.float32).reshape(NCO, P).T),
            ln2w_col=np.ascontiguousarray(
                np.asarray(ln2_w, np.float32).reshape(NCO, P).T),
            bfc2_col=np.ascontiguousarray(
                np.asarray(b_fc2, np.float32).reshape(NCO, P).T),
        ))

    in_maps = []
    for core in range(2 * B):
        s, p = core // 2, core % 2
        xs = x[s]                                     # [T, C]
        xsT = xs.T                                    # [C, T]
        xT = np.ascontiguousarray(
            xsT.reshape(NCO, P, T).transpose(1, 0, 2)).astype(bfloat16)
        xqh = np.ascontiguousarray(
            (xsT * 0.5).reshape(NCO, P, T).transpose(1, 0, 2)).astype(bfloat16)
        in_maps.append(dict(per_par[p], xT=xT, xqh=xqh))
    return nc, in_maps, x


def kernel(x, ln1_w, w_attn, w_proj, ln2_w, w_fc1, b_fc1, w_fc2, b_fc2):
    nc, in_maps, x = _prep(x, ln1_w, w_attn, w_proj, ln2_w,
                           w_fc1, b_fc1, w_fc2, b_fc2)
    B = x.shape[0]
    res = run_bass_kernel_spmd(nc, in_maps, list(range(2 * B)))
    out = np.empty_like(x)
    for core in range(2 * B):
        s, p = core // 2, core % 2
        r = np.asarray(res.results[core]["out"], np.float32)  # [C, 4*Q4]
        for ci in range(4):
            toks = 512 * ci + 256 * p
            out[s, toks:toks + Q4] = r[:, ci * Q4:(ci + 1) * Q4].T
    return out


# revision 26
# speedup vs baseline: 1.1704x; 1.1574x over previous
"""Trainium2 Bass kernel v2 for the dense transformer block (B=4,T=2048,C=1024,H=16,F=4096).

Sharding: core 2s+p owns sequence s; even cores take q-chunks (0,3), odd (1,2)
(identical SPMD program; causality via host-supplied multiplicative masks).

v2 design vs baseline:
- All inputs host-transposed (xT [c,t]); LayerNorm stats computed via ones-matmul
  (float32r) which also broadcasts mu/var across all partitions -> zero PE transposes.
- fp8e4 (scale 16) weights+activations with DoubleRow matmuls (2 contraction
  tiles per instruction) for QKV / AV / proj / fc1 / fc2.
- Scores run 2 heads concurrently in the PE array halves (row tiling via
  base_partition 0/64).
- Softmax normalization deferred: denominators DMA'd to DRAM, one bulk
  reciprocal + stride-0 broadcast DMA, applied after each ci chunk.
- PE work (V part2 / proj / ln2-stats / fc1 / fc2 of the first q-chunk) is
  emitted as micro-blocks interleaved between attention score groups so the
  in-order PE stream fills scalar-engine exp stalls.
- Per-q-chunk tensors (y_stage/yT/xoT/h2T/a1/xq) hold one 512-column chunk and
  are sequentially overwritten to stay under the SBUF cap.
"""
import sys, types
import numpy as np
import ml_dtypes


def _install_hooks():
    try:
        import antenv
        if "antenv.axon_hooks" not in sys.modules:
            m = types.ModuleType("antenv.axon_hooks")
            m._hook = None
            m.set_axon_ntff_profile_hook = lambda h: setattr(m, "_hook", h)
            m.get_axon_ntff_profile_hook = lambda: m._hook
            sys.modules["antenv.axon_hooks"] = m
            antenv.axon_hooks = m
    except Exception:
        pass
_install_hooks()

import concourse.bass as bass
import concourse.tile as tile
from concourse import mybir, bacc
from concourse.bass_utils import run_bass_kernel_spmd

BF16 = mybir.dt.bfloat16
U16 = mybir.dt.uint16
F32 = mybir.dt.float32
F32R = mybir.dt.float32r
FP8 = mybir.dt.float8e4
DRM = mybir.MatmulPerfMode.DoubleRow
AT = mybir.AluOpType
AF = mybir.ActivationFunctionType
bfloat16 = ml_dtypes.bfloat16
f8e4 = ml_dtypes.float8_e4m3

T, C, H, D, F = 2048, 1024, 16, 64, 4096
P, CH = 128, 512
NCO = C // P            # 8 feature tiles
QT = 1024               # q tokens per core
NKT = (8, 16)
SA = 16.0               # activation scale
SW = 16.0               # weight scale
EXPSC = 0.125 / (SA * SW)
EPS = 1e-5

# per-stage fp8 toggles (False -> bf16, plain matmuls)
CFG = dict(qkv=True, scores=True, av=True, proj=True, fc1=False, fc2=False)

_cache = {}


def build_nc(apply_ln1w, apply_ln2w, add_bfc1, add_bfc2):
    nc = bacc.Bacc()
    wdt = {s: (FP8 if CFG[s] else BF16) for s in CFG}
    xT_d = nc.declare_dram_parameter("xT", [P, NCO, T], BF16, isOutput=False)
    xqT_d = nc.declare_dram_parameter("xqT", [P, NCO, QT], F32, isOutput=False)
    xqb_d = nc.declare_dram_parameter("xqb", [P, NCO, QT], BF16, isOutput=False)
    wa_d = nc.declare_dram_parameter("wa_r", [P, NCO, 3 * C], wdt['qkv'], isOutput=False)
    wp_d = nc.declare_dram_parameter("wp_r", [P, NCO, C], wdt['proj'], isOutput=False)
    w1_d = nc.declare_dram_parameter("w1_r", [P, NCO, F], wdt['fc1'], isOutput=False)
    w2_d = nc.declare_dram_parameter("w2_r", [P, F // P, C], wdt['fc2'], isOutput=False)
    mA_d = nc.declare_dram_parameter("maskA", [P, 8, CH], wdt['av'], isOutput=False)
    mB_d = nc.declare_dram_parameter("maskB", [P, 8, CH], wdt['av'], isOutput=False)
    ln1w_d = nc.declare_dram_parameter("ln1w_col", [P, NCO], F32, isOutput=False)
    ln2w_d = nc.declare_dram_parameter("ln2w_col", [P, NCO], F32, isOutput=False)
    b1_d = nc.declare_dram_parameter("bfc1_col", [P, F // P], F32, isOutput=False)
    b2_d = nc.declare_dram_parameter("bfc2_col", [P, NCO], F32, isOutput=False)
    out_d = nc.declare_dram_parameter("out", [C, QT], F32, isOutput=True)
    out_r = out_d.rearrange("(ct p) q -> p ct q", p=P)

    with tile.TileContext(nc) as tc:
        with tc.tile_pool(name="consts", bufs=1) as consts, \
             tc.tile_pool(name="persist", bufs=1) as persist, \
             tc.tile_pool(name="dram", bufs=4, space="DRAM") as dram:
            onesb_sb = consts.tile([P, P], BF16)
            nc.vector.memset(onesb_sb[:], 1.0)
            eps_sb = consts.tile([P, 1], F32)
            nc.vector.memset(eps_sb[:], EPS / (SA * SA))
            mA_sb = consts.tile([P, 8, CH], wdt['av'])
            nc.sync.dma_start(out=mA_sb[:], in_=mA_d[:])
            mB_sb = consts.tile([P, 8, CH], wdt['av'])
            nc.sync.dma_start(out=mB_sb[:], in_=mB_d[:])
            ln1w_sb = consts.tile([P, NCO], F32)
            if apply_ln1w:
                nc.sync.dma_start(out=ln1w_sb[:], in_=ln1w_d[:])
            ln2w_sb = consts.tile([P, NCO], F32)
            if apply_ln2w:
                nc.sync.dma_start(out=ln2w_sb[:], in_=ln2w_d[:])
            b1_sb = consts.tile([P, F // P], F32)
            if add_bfc1:
                nc.sync.dma_start(out=b1_sb[:], in_=b1_d[:])
            else:
                nc.vector.memset(b1_sb[:], 0.0)
            b2_sb = consts.tile([P, NCO], F32)
            if add_bfc2:
                nc.sync.dma_start(out=b2_sb[:], in_=b2_d[:])

            # persistent activations (y_stage/yT/xq/xoT/h2T/a1 hold ONE 512-col
            # q-chunk and are sequentially overwritten)
            kT = persist.tile([P, NCO, T], wdt['scores'])
            qT = persist.tile([P, NCO, QT], wdt['scores'])
            v_sb = persist.tile([P, T // P, H, D + 1], wdt['av'])
            nc.vector.memset(v_sb[:, :, :, D:D + 1], 1.0)
            y_stage = persist.tile([P, NCO, CH], BF16)
            yT = persist.tile([P, NCO, CH], wdt['proj'])
            xq_half = persist.tile([P, NCO, CH], F32)
            xoT = persist.tile([P, NCO, CH], BF16)
            h2T = persist.tile([P, NCO, CH], wdt['fc1'])
            a1 = persist.tile([P, F // P, CH], wdt['fc2'])
            den_d = dram.tile([H, QT], BF16)
            deni_d = dram.tile([H, QT], BF16)

            def load_xq(qc):
                nc.sync.dma_start(out=xq_half[:], in_=xqT_d[:, :, qc * CH:(qc + 1) * CH])

            # ---------------- layernorm via stats-matmuls ----------------
            def ln_emit(p_ln, ps_ln, src_ap_fn, h_out, w_sb, apply_w, col0, ptag, pbufs,
                        f32src=True, offload=False):
                """src_ap_fn(co) -> [P, CH] AP for this 512-token chunk.
                Writes h_out[:, co, col0:col0+CH] = SA*ln(x).
                Stats matmuls run in bf16 (f32 sources are cast per-co)."""
                ps_mu = ps_ln.tile([P, CH], F32, tag=ptag, bufs=pbufs)
                ps_sq = ps_ln.tile([P, CH], F32, tag=ptag, bufs=pbufs)
                for co in range(NCO):
                    if f32src:
                        xb = p_ln.tile([P, CH], BF16, tag="xb", bufs=3)
                        nc.vector.tensor_copy(out=xb[:], in_=src_ap_fn(co))
                        bsrc = xb[:]
                    else:
                        bsrc = src_ap_fn(co)
                    nc.tensor.matmul(ps_mu[:], onesb_sb[:], bsrc,
                                     start=(co == 0), stop=(co == NCO - 1))
                    sq = p_ln.tile([P, CH], BF16, tag="sq", bufs=3)
                    # in phase 1 the DVE is the binding engine and Scalar is
                    # mostly idle, so offload the square there; during the
                    # attention phase Scalar is saturated by exp -> keep DVE
                    if offload:
                        nc.scalar.activation(out=sq[:], in_=src_ap_fn(co), func=AF.Square)
                    else:
                        nc.vector.tensor_mul(out=sq[:], in0=src_ap_fn(co), in1=src_ap_fn(co))
                    nc.tensor.matmul(ps_sq[:], onesb_sb[:], sq[:],
                                     start=(co == 0), stop=(co == NCO - 1))
                mu = p_ln.tile([P, CH], F32, tag="mu", bufs=1)
                nc.vector.tensor_scalar_mul(out=mu[:], in0=ps_mu[:], scalar1=1.0 / C)
                m2 = p_ln.tile([P, CH], F32, tag="m2", bufs=1)
                nc.vector.tensor_mul(out=m2[:], in0=mu[:], in1=mu[:])
                var = p_ln.tile([P, CH], F32, tag="var", bufs=1)
                nc.vector.scalar_tensor_tensor(
                    out=var[:], in0=ps_sq[:], scalar=1.0 / C, in1=m2[:],
                    op0=AT.mult, op1=AT.subtract)
                sd = p_ln.tile([P, CH], F32, tag="sd", bufs=1)
                nc.scalar.activation(out=sd[:], in_=var[:], func=AF.Sqrt,
                                     scale=1.0 / (SA * SA), bias=eps_sb[:])
                rstd = p_ln.tile([P, CH], F32, tag="rstd", bufs=1)
                nc.vector.reciprocal_approx_fast(out=rstd[:], in_=sd[:])  # = SA/sqrt(var+eps)
                for co in range(NCO):
                    dd = p_ln.tile([P, CH], F32, tag="dd", bufs=2)
                    eng_sub = nc.gpsimd if offload else nc.vector
                    eng_sub.tensor_sub(out=dd[:], in0=src_ap_fn(co), in1=mu[:])
                    if apply_w:
                        nc.vector.tensor_mul(out=dd[:], in0=dd[:], in1=rstd[:])
                        nc.vector.tensor_scalar_mul(
                            out=h_out[:, co, col0:col0 + CH], in0=dd[:],
                            scalar1=w_sb[:, co:co + 1])
                    else:
                        nc.vector.tensor_mul(out=h_out[:, co, col0:col0 + CH],
                                             in0=dd[:], in1=rstd[:])

            # ---------------- attention emitter (with PE micro-fillers) ----------
            dr_av = CFG['av']

            def emit_attn(ci, hp, p_pt, p_yt, ps_st, ps_av, drain, stb=2):
                n_kt = NKT[ci]
                pts = [p_pt.tile([P, 16, CH], wdt['av'], tag="pt", bufs=2, name=f"pt{ci}_{hp}_{i}")
                       for i in range(2)]
                for ktp in range(n_kt // 2):
                    sps = [ps_st.tile([P, 2 * CH], F32, tag="st", bufs=stb, name=f"sp{ci}_{hp}_{ktp}_{i}")
                           for i in range(2)]
                    for k2 in range(2):
                        kt = 2 * ktp + k2
                        for h2 in range(2):
                            r0 = h2 * D
                            nc.tensor.matmul(
                                sps[h2][:, k2 * CH:(k2 + 1) * CH],
                                kT[r0:r0 + D, hp, kt * P:(kt + 1) * P],
                                qT[r0:r0 + D, hp, ci * CH:(ci + 1) * CH],
                                start=True, stop=True)
                    for h2 in range(2):
                        nc.scalar.activation(
                            out=pts[h2].rearrange("p k a -> p (k a)")[:, 2 * ktp * CH:(2 * ktp + 2) * CH],
                            in_=sps[h2][:], func=AF.Exp, scale=EXPSC)
                    drain()
                # byte-mask via uint16 bitwise AND (2-byte dtype -> DVE 2x_1p mode,
                # and 2 fp8 lanes per element -> 4x fewer cycles than fp8 mult)
                for h2 in range(2):
                    if ci == 0:
                        pv = pts[h2][:, 0:8, :].bitcast(U16)
                        nc.vector.tensor_tensor(out=pv, in0=pv,
                                                in1=mA_sb[:].bitcast(U16),
                                                op=AT.bitwise_and)
                    else:
                        pv = pts[h2][:, 8:16, :].bitcast(U16)
                        nc.vector.tensor_tensor(out=pv, in0=pv,
                                                in1=mB_sb[:].bitcast(U16),
                                                op=AT.bitwise_and)
                apvs = [ps_av.tile([D + 1, CH], F32, tag="av", bufs=2, name=f"av{ci}_{hp}_{i}")
                        for i in range(2)]
                if dr_av:
                    for t in range(n_kt // 2):
                        for h2 in range(2):
                            nc.tensor.matmul(
                                apvs[h2][:], v_sb[:, 2 * t:2 * t + 2, 2 * hp + h2, :],
                                pts[h2][:, 2 * t:2 * t + 2, :],
                                start=(t == 0), stop=(t == n_kt // 2 - 1),
                                perf_mode=DRM)
                else:
                    for kt in range(n_kt):
                        for h2 in range(2):
                            nc.tensor.matmul(
                                apvs[h2][:], v_sb[:, kt, 2 * hp + h2, :],
                                pts[h2][:, kt, :],
                                start=(kt == 0), stop=(kt == n_kt - 1))
                # evict unnormalized y (bf16) + denominator row to DRAM
                nc.vector.tensor_copy(out=y_stage[0:D, hp, :], in_=apvs[0][0:D, :])
                ytmp = p_yt.tile([D, CH], BF16, tag="ytmp", bufs=2)
                nc.vector.tensor_copy(out=ytmp[:], in_=apvs[1][0:D, :])
                nc.sync.dma_start(out=y_stage[D:P, hp, :], in_=ytmp[:])
                dpr = p_yt.tile([P, 2, CH], BF16, tag="dpr", bufs=1)
                for h2 in range(2):
                    nc.vector.tensor_copy(out=dpr[D:D + 1, h2, :], in_=apvs[h2][D:D + 1, :])
                nc.gpsimd.dma_start(
                    out=den_d[2 * hp:2 * hp + 2, ci * CH:(ci + 1) * CH],
                    in_=dpr[D:D + 1, :, :])
                drain()

            def emit_den_post(ci, p_dn):
                den_sb = p_dn.tile([H, CH], BF16, tag="den", bufs=1)
                nc.sync.dma_start(out=den_sb[:], in_=den_d[:, ci * CH:(ci + 1) * CH])
                deni = p_dn.tile([H, CH], F32, tag="deni", bufs=1)
                nc.vector.reciprocal(out=deni[:], in_=den_sb[:])
                denib = p_dn.tile([H, CH], BF16, tag="denib", bufs=1)
                nc.vector.tensor_copy(out=denib[:], in_=deni[:])
                nc.sync.dma_start(out=deni_d[:, ci * CH:(ci + 1) * CH], in_=denib[:])

                for hp in range(NCO):
                    dbc = p_dn.tile([P, CH], BF16, tag="dbc", bufs=2)
                    eng = nc.gpsimd if hp % 2 == 0 else nc.sync
                    for h2 in range(2):
                        row = deni_d[2 * hp + h2:2 * hp + h2 + 1, ci * CH:(ci + 1) * CH]
                        eng.dma_start(
                            out=dbc[h2 * D:(h2 + 1) * D, :],
                            in_=bass.AP(tensor=row.tensor, offset=row.offset,
                                        ap=[[0, D]] + [list(a) for a in row.ap]))
                    nc.vector.tensor_mul(out=yT[:, hp, :], in0=y_stage[:, hp, :],
                                         in1=dbc[:])

            def make_drain(queue, n_calls):
                state = {'calls': 0, 'done': 0}
                total = len(queue)

                def drain():
                    state['calls'] += 1
                    want = (total * state['calls']) // n_calls
                    while state['done'] < min(want, total):
                        queue[state['done']]()
                        state['done'] += 1
                return drain

            # ============ phase A: ln1 interleaved with K/Q, then V ============
            with tc.tile_pool(name="p_h1", bufs=1) as p_h1:
                h1T = p_h1.tile([P, NCO, T], wdt['qkv'])
                h1qT = p_h1.tile([P, NCO, QT], wdt['qkv'])
                dr_qkv = CFG['qkv']
                ncj = 4 if dr_qkv else 8

                def h1_ap(src, j, tcol, width):
                    if dr_qkv:
                        return src[:, 2 * j:2 * j + 2, tcol:tcol + width]
                    return src[:, j, tcol:tcol + width]

                with tc.tile_pool(name="p_wkq", bufs=1) as p_wkq, \
                     tc.tile_pool(name="p_ln", bufs=1) as p_ln, \
                     tc.tile_pool(name="p_xt", bufs=1) as p_xt, \
                     tc.tile_pool(name="ps_lnkq", bufs=1, space="PSUM") as ps_lnkq:
                    wkq = p_wkq.tile([P, NCO, 2 * C], wdt['qkv'])
                    nc.gpsimd.dma_start(out=wkq[:], in_=wa_d[:, :, 0:2 * C])

                    def wkq_ap(j, fcol, width):
                        if dr_qkv:
                            return wkq[:, 2 * j:2 * j + 2, fcol:fcol + width]
                        return wkq[:, j, fcol:fcol + width]

                    def k_block(tcx):
                        for hp in range(NCO):
                            ps = ps_lnkq.tile([P, CH], F32, tag="kqv", bufs=3,
                                              name=f"kps{tcx}_{hp}")
                            for j in range(ncj):
                                nc.tensor.matmul(
                                    ps[:], wkq_ap(j, C + hp * P, P),
                                    h1_ap(h1T, j, tcx * CH, CH),
                                    start=(j == 0), stop=(j == ncj - 1),
                                    perf_mode=DRM if dr_qkv else None)
                            nc.scalar.activation(
                                out=kT[:, hp, tcx * CH:(tcx + 1) * CH], in_=ps[:],
                                func=AF.Copy, scale=1.0 / SA)

                    def q_block(qc):
                        for hp in range(NCO):
                            ps = ps_lnkq.tile([P, CH], F32, tag="kqv", bufs=3,
                                              name=f"qps{qc}_{hp}")
                            for j in range(ncj):
                                nc.tensor.matmul(
                                    ps[:], wkq_ap(j, hp * P, P),
                                    h1_ap(h1qT, j, qc * CH, CH),
                                    start=(j == 0), stop=(j == ncj - 1),
                                    perf_mode=DRM if dr_qkv else None)
                            nc.scalar.activation(
                                out=qT[:, hp, qc * CH:(qc + 1) * CH], in_=ps[:],
                                func=AF.Copy, scale=1.0 / SA)

                    for tcx in range(T // CH):
                        xt = p_xt.tile([P, NCO, CH], BF16, tag="xt", bufs=2,
                                       name=f"xt_{tcx}")
                        nc.sync.dma_start(out=xt[:], in_=xT_d[:, :, tcx * CH:(tcx + 1) * CH])
                        ln_emit(p_ln, ps_lnkq, lambda co: xt[:, co, :], h1T,
                                ln1w_sb, apply_ln1w, tcx * CH, "lnp", 4, f32src=False)
                        k_block(tcx)
                    for qc in range(2):
                        xqb = p_xt.tile([P, NCO, CH], BF16, tag="xt", bufs=2,
                                        name=f"xqb_{qc}")
                        nc.sync.dma_start(out=xqb[:], in_=xqb_d[:, :, qc * CH:(qc + 1) * CH])
                        ln_emit(p_ln, ps_lnkq, lambda co: xqb[:, co, :], h1qT,
                                ln1w_sb, apply_ln1w, qc * CH, "lnp", 4, f32src=False)
                        q_block(qc)

                with tc.tile_pool(name="p_wv", bufs=1) as p_wv, \
                     tc.tile_pool(name="ps_v", bufs=1, space="PSUM") as ps_v:
                    wv = p_wv.tile([P, NCO, C], wdt['qkv'])
                    nc.gpsimd.dma_start(out=wv[:], in_=wa_d[:, :, 2 * C:3 * C])

                    def emit_v_one(tt, vc):
                        ps = ps_v.tile([P, CH], F32, tag="vps", bufs=2,
                                       name=f"vps{tt}_{vc}")
                        for j in range(ncj):
                            if dr_qkv:
                                lw = h1T[:, 2 * j:2 * j + 2, tt * P:(tt + 1) * P]
                                rh = wv[:, 2 * j:2 * j + 2, vc * CH:(vc + 1) * CH]
                            else:
                                lw = h1T[:, j, tt * P:(tt + 1) * P]
                                rh = wv[:, j, vc * CH:(vc + 1) * CH]
                            nc.tensor.matmul(ps[:], lw, rh, start=(j == 0),
                                             stop=(j == ncj - 1),
                                             perf_mode=DRM if dr_qkv else None)
                        nc.scalar.activation(
                            out=v_sb[:, tt, vc * 8:(vc + 1) * 8, 0:D],
                            in_=ps.rearrange("p (h d) -> p h d", d=D),
                            func=AF.Copy, scale=1.0 / SA)

                    for tt in range(8):
                        for vc in range(2):
                            emit_v_one(tt, vc)

                    # attention ci0 with V part-2 interleaved as PE fillers
                    with tc.tile_pool(name="p_pt", bufs=1) as p_pt, \
                         tc.tile_pool(name="p_yt", bufs=1) as p_yt, \
                         tc.tile_pool(name="ps_st", bufs=1, space="PSUM") as ps_st, \
                         tc.tile_pool(name="ps_av", bufs=1, space="PSUM") as ps_av:
                        v2q = [(lambda tt=tt, vc=vc: emit_v_one(tt, vc))
                               for tt in range(8, 16) for vc in range(2)]
                        drain0 = make_drain(v2q, NCO * (NKT[0] // 2 + 1))
                        for hp in range(NCO):
                            emit_attn(0, hp, p_pt, p_yt, ps_st, ps_av, drain0, stb=2)

            # wkq/wv/h1T freed
            # ============ den-post ci0, then attention ci1 + qc0 MLP fillers ====
            dr_proj, dr_fc1, dr_fc2 = CFG['proj'], CFG['fc1'], CFG['fc2']

            with tc.tile_pool(name="p_wp", bufs=1) as p_wp, \
                 tc.tile_pool(name="p_dn", bufs=1) as p_dn, \
                 tc.tile_pool(name="p_w12", bufs=1) as p_w12, \
                 tc.tile_pool(name="p_ln2", bufs=1) as p_ln2, \
                 tc.tile_pool(name="p_ot", bufs=1) as p_ot:
                wp_sb = p_wp.tile([P, NCO, C], wdt['proj'])
                nc.gpsimd.dma_start(out=wp_sb[:], in_=wp_d[:])

                def emit_proj(qc, cf, psp, psb=2):
                    njp = 4 if dr_proj else 8
                    ps = psp.tile([P, CH], F32, tag="fill", bufs=psb)
                    for j in range(njp):
                        if dr_proj:
                            lw = wp_sb[:, 2 * j:2 * j + 2, cf * P:(cf + 1) * P]
                            rh = yT[:, 2 * j:2 * j + 2, :]
                        else:
                            lw = wp_sb[:, j, cf * P:(cf + 1) * P]
                            rh = yT[:, j, :]
                        nc.tensor.matmul(ps[:], lw, rh, start=(j == 0),
                                         stop=(j == njp - 1),
                                         perf_mode=DRM if dr_proj else None)
                    nc.vector.scalar_tensor_tensor(
                        out=xoT[:, cf, :], in0=ps[:], scalar=1.0 / (SA * SW),
                        in1=xq_half[:, cf, :], op0=AT.mult, op1=AT.add)

                def emit_ln2(qc, psp, psb=2):
                    ln_emit(p_ln2, psp, lambda co: xoT[:, co, :], h2T,
                            ln2w_sb, apply_ln2w, 0, "fill", psb, f32src=False)

                w1h = {}

                def emit_fc1_load(fh):
                    w1t = p_w12.tile([P, NCO, F // 8], wdt['fc1'], tag="w1", bufs=2)
                    nc.gpsimd.dma_start(out=w1t[:], in_=w1_d[:, :, fh * (F // 8):(fh + 1) * (F // 8)])
                    w1h['t'] = w1t

                def emit_fc1(qc, fh, ft, psp, psb=2, relu_act=False):
                    njp = 4 if dr_fc1 else 8
                    w1t = w1h['t']
                    fg = fh * 4 + ft
                    ps = psp.tile([P, CH], F32, tag="fill", bufs=psb)
                    for j in range(njp):
                        if dr_fc1:
                            lw = w1t[:, 2 * j:2 * j + 2, ft * P:(ft + 1) * P]
                            rh = h2T[:, 2 * j:2 * j + 2, :]
                        else:
                            lw = w1t[:, j, ft * P:(ft + 1) * P]
                            rh = h2T[:, j, :]
                        nc.tensor.matmul(ps[:], lw, rh, start=(j == 0),
                                         stop=(j == njp - 1),
                                         perf_mode=DRM if dr_fc1 else None)
                    if add_bfc1 or relu_act:
                        nc.scalar.activation(
                            out=a1[:, fg, :], in_=ps[:], func=AF.Relu,
                            scale=1.0 / SW, bias=b1_sb[:, fg:fg + 1])
                    else:
                        nc.vector.tensor_scalar(
                            out=a1[:, fg, :], in0=ps[:], scalar1=1.0 / SW,
                            scalar2=0.0, op0=AT.mult, op1=AT.max)

                w2h = {}

                def emit_fc2_load(ct, wpool):
                    w2t = wpool.tile([P, F // P, P], wdt['fc2'], tag="w2", bufs=2)
                    nc.gpsimd.dma_start(out=w2t[:], in_=w2_d[:, :, ct * P:(ct + 1) * P])
                    w2h[ct] = w2t

                def emit_fc2(qc, ct, psp, wpool, psb=2):
                    njp = 16 if dr_fc2 else 32
                    w2t = w2h.pop(ct)
                    ps = psp.tile([P, CH], F32, tag="fill", bufs=psb)
                    for j in range(njp):
                        if dr_fc2:
                            lw = w2t[:, 2 * j:2 * j + 2, :]
                            rh = a1[:, 2 * j:2 * j + 2, :]
                        else:
                            lw = w2t[:, j, :]
                            rh = a1[:, j, :]
                        nc.tensor.matmul(ps[:], lw, rh, start=(j == 0),
                                         stop=(j == njp - 1),
                                         perf_mode=DRM if dr_fc2 else None)
                    ot = p_ot.tile([P, CH], F32, tag="ot", bufs=2)
                    if add_bfc2:
                        nc.vector.tensor_scalar(
                            out=ps[:], in0=ps[:], scalar1=1.0 / (SA * SW),
                            scalar2=b2_sb[:, ct:ct + 1], op0=AT.mult, op1=AT.add)
                        nc.vector.tensor_add(out=ot[:], in0=ps[:], in1=xoT[:, ct, :])
                    else:
                        nc.vector.scalar_tensor_tensor(
                            out=ot[:], in0=ps[:], scalar=1.0 / (SA * SW),
                            in1=xoT[:, ct, :], op0=AT.mult, op1=AT.add)
                    nc.sync.dma_start(out=out_r[:, ct, qc * CH:(qc + 1) * CH], in_=ot[:])

                # qc0 pipeline: den-post(0) first (y_stage/yT reused by ci1)
                emit_den_post(0, p_dn)
                load_xq(0)

                with tc.tile_pool(name="p_pt2", bufs=1) as p_pt2, \
                     tc.tile_pool(name="p_yt2", bufs=1) as p_yt2, \
                     tc.tile_pool(name="ps_st2", bufs=1, space="PSUM") as ps_st2, \
                     tc.tile_pool(name="ps_av2", bufs=1, space="PSUM") as ps_av2, \
                     tc.tile_pool(name="ps_fill", bufs=1, space="PSUM") as ps_fill:
                    fillq = [(lambda cf=cf: emit_proj(0, cf, ps_fill)) for cf in range(NCO)]
                    fillq += [lambda: emit_ln2(0, ps_fill)]
                    for fh in range(8):
                        fillq += [lambda fh=fh: emit_fc1_load(fh)]
                        fillq += [(lambda fh=fh, ft=ft: emit_fc1(0, fh, ft, ps_fill))
                                  for ft in range(4)]
                    drain1 = make_drain(fillq, NCO * (NKT[1] // 2 + 1))
                    for hp in range(NCO):
                        emit_attn(1, hp, p_pt2, p_yt2, ps_st2, ps_av2, drain1)

                # tail: fc2(qc0) + all of qc1 with a wide psum pool.
                # den_post(1)/xq(1) DMAs are issued first so their round trips
                # hide under the fc2(qc0) matmuls; w2 tiles are prefetched one
                # chunk ahead.
                with tc.tile_pool(name="ps_tail", bufs=1, space="PSUM") as ps_tail, \
                     tc.tile_pool(name="p_w2", bufs=1) as p_w2:
                    emit_den_post(1, p_dn)
                    load_xq(1)
                    emit_fc2_load(0, p_w2)
                    for ct in range(NCO):
                        if ct + 1 < NCO:
                            emit_fc2_load(ct + 1, p_w2)
                        emit_fc2(0, ct, ps_tail, p_w2, psb=6)
                    for cf in range(NCO):
                        emit_proj(1, cf, ps_tail, psb=6)
                    emit_ln2(1, ps_tail, psb=6)
                    for fh in range(8):
                        emit_fc1_load(fh)
                        for ft in range(4):
                            emit_fc1(1, fh, ft, ps_tail, psb=6, relu_act=True)
                    emit_fc2_load(0, p_w2)
                    for ct in range(NCO):
                        if ct + 1 < NCO:
                            emit_fc2_load(ct + 1, p_w2)
                        emit_fc2(1, ct, ps_tail, p_w2, psb=6)

    nc.compile()
    return nc


def _diag_masks():
    f = np.arange(CH)[None, :]
    p = np.arange(P)[:, None]
    d = [(f >= (P * j + p)).astype(np.float32) for j in range(4)]
    one = np.ones((P, CH), np.float32)
    zero = np.zeros((P, CH), np.float32)
    mA_even = np.stack(d + [zero] * 4, 1)
    mA_odd = np.stack([one] * 4 + d, 1)
    mB_even = np.stack([one] * 4 + d, 1)
    mB_odd = np.stack(d + [zero] * 4, 1)
    return (mA_even, mB_even), (mA_odd, mB_odd)


def _wcast(stage, a, scale):
    npdt = f8e4 if CFG[stage] else bfloat16
    return np.ascontiguousarray(np.asarray(a, np.float32) * scale).astype(npdt)


def _prep(x, ln1_w, w_attn, w_proj, ln2_w, w_fc1, b_fc1, w_fc2, b_fc2):
    x = np.asarray(x, np.float32)
    B = x.shape[0]
    apply_ln1w = not np.allclose(ln1_w, 1.0)
    apply_ln2w = not np.allclose(ln2_w, 1.0)
    add_bfc1 = not np.allclose(b_fc1, 0.0)
    add_bfc2 = not np.allclose(b_fc2, 0.0)
    key = (apply_ln1w, apply_ln2w, add_bfc1, add_bfc2)
    if key not in _cache:
        _cache[key] = build_nc(*key)
    nc = _cache[key]

    def r3(w, n):  # [rows, n] -> [p, ro, n]
        w = np.asarray(w, np.float32)
        return w.reshape(w.shape[0] // P, P, n).transpose(1, 0, 2)

    shared = dict(
        wa_r=_wcast('qkv', r3(w_attn, 3 * C), SW),
        wp_r=_wcast('proj', r3(w_proj, C), SW),
        w1_r=_wcast('fc1', r3(w_fc1, F), SW),
        w2_r=_wcast('fc2', r3(w_fc2, C), SW),
        ln1w_col=np.ascontiguousarray(np.asarray(ln1_w, np.float32).reshape(NCO, P).T),
        ln2w_col=np.ascontiguousarray(np.asarray(ln2_w, np.float32).reshape(NCO, P).T),
        bfc1_col=np.ascontiguousarray(np.asarray(b_fc1, np.float32).reshape(F // P, P).T * SA),
        bfc2_col=np.ascontiguousarray(np.asarray(b_fc2, np.float32).reshape(NCO, P).T),
    )
    (mAe, mBe), (mAo, mBo) = _diag_masks()
    mdt = f8e4 if CFG['av'] else bfloat16
    isz = np.dtype(mdt).itemsize

    def _byte_mask(m):  # 0/1 float mask -> 0x00/0xFF bytes viewed as mdt
        b = np.repeat((np.ascontiguousarray(m) > 0).astype(np.uint8) * np.uint8(0xFF),
                      isz, axis=-1)
        return b.view(mdt).reshape(m.shape)

    masks = ((_byte_mask(mAe), _byte_mask(mBe)),
             (_byte_mask(mAo), _byte_mask(mBo)))
    chunks = ((0, 3), (1, 2))
    in_maps = []
    for core in range(2 * B):
        s, par = core // 2, core % 2
        c0, c1 = chunks[par]
        xs = x[s]
        xq = np.concatenate([xs[c0 * CH:(c0 + 1) * CH], xs[c1 * CH:(c1 + 1) * CH]], 0)
        xT = np.ascontiguousarray(xs.T.reshape(NCO, P, T).transpose(1, 0, 2)).astype(bfloat16)
        xqT = np.ascontiguousarray(xq.T.reshape(NCO, P, QT).transpose(1, 0, 2))
        xqb = xqT.astype(bfloat16)
        mA, mB = masks[par]
        in_maps.append(dict(shared, xT=xT, xqT=xqT, xqb=xqb, maskA=mA, maskB=mB))
    return nc, in_maps, chunks, x


def kernel(x, ln1_w, w_attn, w_proj, ln2_w, w_fc1, b_fc1, w_fc2, b_fc2):
    nc, in_maps, chunks, x = _prep(x, ln1_w, w_attn, w_proj, ln2_w,
                                   w_fc1, b_fc1, w_fc2, b_fc2)
    B = x.shape[0]
    res = run_bass_kernel_spmd(nc, in_maps, list(range(2 * B)))
    out = np.empty_like(x)
    for core in range(2 * B):
        s, par = core // 2, core % 2
        c0, c1 = chunks[par]
        r = res.results[core]["out"]       # [C, QT] (c-major)
        rT = np.asarray(r, np.float32).T   # [QT, C]
        out[s, c0 * CH:(c0 + 1) * CH] = rT[0:CH]
        out[s, c1 * CH:(c1 + 1) * CH] = rT[CH:2 * CH]
    return out

